# revision 4
# baseline (speedup 1.0000x reference)
"""Trainium2 Bass kernel for the n-ary span-compose problem (gnn_message_passing).

Strategy v2 (zero cross-core communication, host-planned, no dma_gather):
  The host resolves the full version DAG (which value every compose reads and
  which write wins each output position).  Needed composes form tiny connected
  components, distributed over 8 cores balancing MLP work and embedding-stream
  length (with token-overlap-aware clustering to cut duplication).

  Each core keeps a TRANSPOSED value log resident in SBUF:
      vT[p, s, j] = value_of_slot_s[j*128 + p]   ([128, nslots, 2] bf16)
  slots: 0 = zeros (pad reads), [1, 1+NTOKP) = down-projected embeddings of
  the core's token stream, then compose outputs level by level.

  Phase A: the per-core token stream (compose-read tokens first, base-final
  fillers last) is compacted ON HOST into a dense [NTOKP, 768] bf16 input and
  streamed with xbar transpose DMA (dma_start_transpose -> pre-transposed
  lhsT-ready tiles, no GpSimd descriptor generation).  Down-projection runs
  as a transposed GEMM (lhsT = w_down) writing straight into vT.

  Compose tiles (width 256/128) fetch their 4 operands per compose with
  ap_gather (GpSimd SIMD ucode gather from SBUF along the free dim), sum with
  3 DVE adds (last add writes the bf16 transposed mean), fold the 1/cnt mean
  scale into the GELU's scale argument, and run both MLP layers as transposed
  GEMMs (lhsT = wc1 / wc2) so no PE transposes are needed anywhere.  Outputs
  are copied from PSUM straight into vT.

  vT is dumped to DRAM incrementally; the host assembles the final
  [16, 2048, 256] output from (core, slot) maps.  The filler part of phase A
  is emitted AFTER the compose tiles so the PE stream has no gaps.
"""

import sys
import types
import numpy as np
import ml_dtypes
from contextlib import ExitStack

import concourse.bass as bass
import concourse.bacc as bacc
import concourse.mybir as mybir
import concourse.tile as tile
from concourse.bass_utils import run_bass_kernel_spmd

N_CORES = 8
NPOS = 16 * 2048
NLEV = 3
NSPAN = 4096
VOCAB = 32000
D = 768
CD = 256
HD = 1024
P = 128
F32 = mybir.dt.float32
I32 = mybir.dt.int32
BF16 = mybir.dt.bfloat16
I16 = mybir.dt.int16

ACHUNK = 256      # rows per phase-A stream chunk
WTILE = 256       # composes per supertile (last tile of a level may be 128)


# --------------------------------------------------------------------------
# host planner
# --------------------------------------------------------------------------

def _last_wins(tgt):
    u, first_rev = np.unique(tgt[::-1], return_index=True)
    return u, len(tgt) - 1 - first_rev


def plan(chunk_input_ids, spans_list):
    ids = np.asarray(chunk_input_ids).astype(np.int64).ravel()
    ids = np.where(ids == -100, 0, ids)
    assert ids.size == NPOS

    # ---- version DAG ----
    ver = np.arange(NPOS, dtype=np.int64)
    comp_reads, comp_cnt = [], []
    for l, spans in enumerate(spans_list):
        spans = np.asarray(spans).astype(np.int64)
        mask = spans != -100
        tgt = spans.max(-1) + 1
        idx = np.where(mask, spans, 0)
        rd = np.where(mask, ver[idx], -1)
        comp_reads.append(rd)
        comp_cnt.append(mask.sum(-1))
        u, win = _last_wins(tgt)
        ver[u] = NPOS + l * NSPAN + win
    final_ver = ver

    # ---- liveness ----
    needed = [np.zeros(NSPAN, bool) for _ in range(NLEV)]
    fin_comp = final_ver[final_ver >= NPOS] - NPOS
    for l in range(NLEV):
        needed[l][fin_comp[fin_comp // NSPAN == l] % NSPAN] = True
    for l in range(NLEV - 1, -1, -1):
        rd = comp_reads[l][needed[l]].ravel()
        rd = rd[rd >= NPOS] - NPOS
        for l2 in range(l):
            needed[l2][rd[rd // NSPAN == l2] % NSPAN] = True

    # ---- connected components over comp->comp read edges ----
    parent = {}

    def find(x):
        root = x
        while parent[root] != root:
            root = parent[root]
        while parent[x] != root:
            parent[x], x = root, parent[x]
        return root

    for l in range(NLEV):
        for r in np.nonzero(needed[l])[0]:
            parent[l * NSPAN + r] = l * NSPAN + r
    for l in range(NLEV):
        rows = np.nonzero(needed[l])[0]
        rd = comp_reads[l][rows]
        for i, r in enumerate(rows):
            for v in rd[i]:
                if v >= NPOS:
                    ra, rb = find(l * NSPAN + int(r)), find(int(v - NPOS))
                    if ra != rb:
                        parent[ra] = rb

    comps_by_root = {}
    for node in parent:
        comps_by_root.setdefault(find(node), []).append(node)

    # ---- group metadata: per-level comp counts + compose-read token sets ----
    groups = []
    for g in comps_by_root.values():
        per_lvl = np.zeros(NLEV, np.int64)
        toks = set()
        for uid in g:
            l = uid // NSPAN
            per_lvl[l] += 1
            for v in comp_reads[l][uid % NSPAN]:
                v = int(v)
                if 0 <= v < NPOS:
                    toks.add(int(ids[v]))
        groups.append((g, per_lvl, toks))

    # ---- greedy assignment: balance MLP comps + token stream, cluster by
    #      token overlap (newtok term) ----
    WC, WT = 18.5, 7.0   # ~ns per compose (MLP) / per streamed token row
    comp_core = {}
    compload = np.zeros((N_CORES, NLEV))
    tokload = np.zeros(N_CORES)
    tok_sets = [set() for _ in range(N_CORES)]
    order = sorted(range(len(groups)),
                   key=lambda i: -(len(groups[i][0]) * 4 + len(groups[i][2])))
    for gi in order:
        g, per_lvl, toks = groups[gi]
        best, bestc = None, 0
        for c in range(N_CORES):
            newtok = sum(1 for t in toks if t not in tok_sets[c])
            # weighted work after assignment + soft max-balance on both axes
            score = (WC * (compload[c].sum() + per_lvl.sum())
                     + WT * (tokload[c] + newtok)
                     + 0.25 * WC * (compload[c] + per_lvl).max())
            if best is None or score < best:
                best, bestc = score, c
        c = bestc
        for uid in g:
            comp_core[uid] = c
        compload[c] += per_lvl
        tokload[c] += sum(1 for t in toks if t not in tok_sets[c])
        tok_sets[c].update(toks)

    # ---- base-final tokens: canonical core (prefer one that has it) ----
    is_comp_final = final_ver >= NPOS
    base_pos = np.nonzero(~is_comp_final)[0]
    tok_canon = {}
    filler = [[] for _ in range(N_CORES)]
    fill_load = np.array([len(s) for s in tok_sets], np.int64)
    for p in base_pos:
        t = int(ids[p])
        if t in tok_canon:
            continue
        for c in range(N_CORES):
            if t in tok_sets[c]:
                tok_canon[t] = c
                break
        else:
            c = int(np.argmin(fill_load))
            tok_canon[t] = c
            filler[c].append(t)
            fill_load[c] += 1

    # ---- per-core streams / slots / tiles ----
    def rup(x, m):
        return -(-int(x) // m) * m

    NTOK = [0] * N_CORES
    core_tok_list = []
    core_tok_slot = []
    for c in range(N_CORES):
        # token first-use order (sweep comps in (level, row) order)
        lst, seen = [], set()
        for l in range(NLEV):
            rows = sorted(uid % NSPAN for uid, cc in comp_core.items()
                          if cc == c and uid // NSPAN == l)
            for r in rows:
                for v in comp_reads[l][r]:
                    v = int(v)
                    if 0 <= v < NPOS:
                        t = int(ids[v])
                        if t not in seen:
                            seen.add(t)
                            lst.append(t)
        lst.extend(filler[c])
        core_tok_list.append(lst)
        core_tok_slot.append({t: 1 + i for i, t in enumerate(lst)})
        NTOK[c] = len(lst)

    NTOKP = rup(max(NTOK), ACHUNK)
    # number of leading A-chunks that contain compose-read tokens (shared)
    nctok = max(len(s) for s in tok_sets)
    A1_CHUNKS = -(-nctok // ACHUNK)
    A_CHUNKS = NTOKP // ACHUNK

    ncmp = np.zeros((N_CORES, NLEV), np.int64)
    for uid, c in comp_core.items():
        ncmp[c, uid // NSPAN] += 1
    NC = [int(rup(ncmp[:, l].max(), P)) for l in range(NLEV)]
    lvl_base = []
    b = 1 + NTOKP
    for l in range(NLEV):
        lvl_base.append(b)
        b += NC[l]
    nslots = b
    assert nslots < 32768

    # tile widths per level (shared across cores)
    tiles = []   # list of (level, base_slot, W)
    for l in range(NLEV):
        off = 0
        while off < NC[l]:
            w = WTILE if NC[l] - off >= WTILE else P
            tiles.append((l, lvl_base[l] + off, w))
            off += w

    inv_vals = set()
    core_rd = []       # per-core concatenated idx arrays (k-major per tile)
    core_bounds = []   # per-core per-tile bound
    core_slot_of_comp = []
    for c in range(N_CORES):
        slot_of_tok = core_tok_slot[c]
        slot_of_comp = {}
        rd_all = []
        bounds = []

        def vslot(v):
            v = int(v)
            if v == -1:
                return 0
            if v < NPOS:
                return slot_of_tok[int(ids[v])]
            return slot_of_comp[v - NPOS]

        for l in range(NLEV):
            rows = sorted(uid % NSPAN for uid, cc in comp_core.items()
                          if cc == c and uid // NSPAN == l)
            # sort by max read slot so early tiles depend on early slots
            def row_bound(r):
                return max((vslot(v) for v in comp_reads[l][r]), default=0)
            rows = sorted(rows, key=lambda r: (row_bound(r), r))
            for i, r in enumerate(rows):
                slot_of_comp[l * NSPAN + int(r)] = lvl_base[l] + i
                inv_vals.add(1.0 / max(int(comp_cnt[l][r]), 1))
            # idx arrays per tile of this level
            rs = np.zeros((NC[l], 4), np.int64)
            for i, r in enumerate(rows):
                for k in range(4):
                    rs[i, k] = vslot(comp_reads[l][r, k])
            off = 0
            for (tl, tbase, w) in tiles:
                if tl != l:
                    continue
                blk = rs[off:off + w]          # [w, 4]
                rd_all.append(blk.T.reshape(-1))   # k-major [4*w]
                bounds.append(max(1, int(blk.max()) + 1))
                off += w
        core_rd.append(np.concatenate(rd_all))
        core_bounds.append(bounds)
        core_slot_of_comp.append(slot_of_comp)

    bounds = tuple(max(core_bounds[c][i] for c in range(N_CORES))
                   for i in range(len(tiles)))
    for c in range(N_CORES):
        for i, (_, tbase, w) in enumerate(tiles):
            assert bounds[i] <= tbase

    if not inv_vals:
        inv_vals = {0.25}
    assert len(inv_vals) == 1, f"non-uniform span counts {inv_vals}"
    inv_uniform = float(inv_vals.pop())

    # ---- output assembly maps ----
    pos_core = np.empty(NPOS, np.int64)
    pos_slot = np.empty(NPOS, np.int64)
    for p in range(NPOS):
        v = int(final_ver[p])
        if v < NPOS:
            t = int(ids[v])
            c = tok_canon[t]
            pos_core[p] = c
            pos_slot[p] = core_tok_slot[c][t]
        else:
            c = comp_core[v - NPOS]
            pos_core[p] = c
            pos_slot[p] = core_slot_of_comp[c][v - NPOS]

    cores = []
    for c in range(N_CORES):
        cores.append(dict(tok_ids=core_tok_list[c], rd=core_rd[c]))
    meta = dict(NTOKP=NTOKP, A1_CHUNKS=A1_CHUNKS, A_CHUNKS=A_CHUNKS,
                tiles=tuple(tiles), bounds=bounds, nslots=nslots,
                inv=inv_uniform, pos_core=pos_core, pos_slot=pos_slot)
    return cores, meta


def wrap_idx16(idx):
    """[n] -> [128, n/16] int16 layout for gpsimd gathers (i -> (i%16, i//16))."""
    idx = np.asarray(idx, np.int64)
    n = len(idx)
    assert n % 16 == 0 and idx.max() < 32768 and idx.min() >= 0
    w = idx.reshape(n // 16, 16).T.astype(np.int16)
    return np.tile(w, (8, 1))


# --------------------------------------------------------------------------
# bass program
# --------------------------------------------------------------------------

def build_bass(NTOKP, A1_CHUNKS, A_CHUNKS, tiles, bounds, nslots,
               has_bd, has_b1, has_b2, inv):
    nc = bacc.Bacc("TRN2", target_bir_lowering=False, debug=False,
                   num_devices=N_CORES, num_swdge_queues=4)

    emb_s = nc.dram_tensor("emb_s", [NTOKP, D], BF16, kind="ExternalInput")
    w_down = nc.dram_tensor("w_down", [D, CD], BF16, kind="ExternalInput")
    b_down = nc.dram_tensor("b_down", [1, CD], F32, kind="ExternalInput")
    wc1 = nc.dram_tensor("wc1", [CD, HD], BF16, kind="ExternalInput")
    bc1 = nc.dram_tensor("bc1", [1, HD], F32, kind="ExternalInput")
    wc2 = nc.dram_tensor("wc2", [HD, CD], BF16, kind="ExternalInput")
    bc2 = nc.dram_tensor("bc2", [1, CD], F32, kind="ExternalInput")
    tot_idx = sum(4 * w for (_, _, w) in tiles)
    rd_idx = nc.dram_tensor("rd_idx", [P, tot_idx // 16], I16,
                            kind="ExternalInput")
    vlogT = nc.dram_tensor("vlogT", [P, nslots, 2], BF16,
                           kind="ExternalOutput")

    with tile.TileContext(nc) as tc, ExitStack() as ctx:
        cst = ctx.enter_context(tc.tile_pool(name="cst", bufs=1))
        sb = ctx.enter_context(tc.tile_pool(name="sb", bufs=3))
        ps = ctx.enter_context(tc.tile_pool(name="ps", bufs=2, space="PSUM"))

        rd_sb = cst.tile([P, tot_idx // 16], I16)
        nc.scalar.dma_start(rd_sb[:], rd_idx[:])

        # weights as lhsT chunks (natural [K, M] layout)
        w_sb = cst.tile([P, D // P, CD], BF16)
        for k in range(D // P):
            nc.scalar.dma_start(w_sb[:, k, :], w_down[k * P:(k + 1) * P, :])
        wc1_sb = cst.tile([P, CD // P, HD], BF16)
        for k in range(CD // P):
            nc.scalar.dma_start(wc1_sb[:, k, :], wc1[k * P:(k + 1) * P, :])
        wc2_sb = cst.tile([P, HD // P, CD], BF16)
        for k in range(HD // P):
            nc.scalar.dma_start(wc2_sb[:, k, :], wc2[k * P:(k + 1) * P, :])

        ones1 = cst.tile([1, WTILE], F32)
        nc.vector.memset(ones1[:], 1.0)
        bd_sb = cst.tile([1, CD], F32)
        nc.scalar.dma_start(bd_sb[:], b_down[:])
        bc1_sb = cst.tile([1, HD], F32)
        nc.scalar.dma_start(bc1_sb[:], bc1[:])
        bc2_sb = cst.tile([1, CD], F32)
        nc.scalar.dma_start(bc2_sb[:], bc2[:])

        # the SBUF-resident transposed value log
        vT = cst.tile([P, nslots, 2], BF16)
        nc.vector.memset(vT[:, 0:1, :], 0.0)
        nc.sync.dma_start(vlogT[:, 0:1, :], vT[:, 0:1, :])

        def a_chunk(i):
            """stream + down-project rows [i*ACHUNK, (i+1)*ACHUNK) into vT."""
            s0 = 1 + i * ACHUNK
            embT = sb.tile([P, D // P, ACHUNK], BF16, tag="embT", bufs=4)
            nc.sync.dma_start_transpose(
                embT[:], emb_s[i * ACHUNK:(i + 1) * ACHUNK, :])
            for j in range(CD // P):
                acc = ps.tile([P, ACHUNK], F32, tag="acc", bufs=2)
                if has_bd:
                    nc.tensor.matmul(acc[:], lhsT=bd_sb[:, j * P:(j + 1) * P],
                                     rhs=ones1[:, 0:ACHUNK],
                                     start=True, stop=False)
                for k in range(D // P):
                    nc.tensor.matmul(acc[:], lhsT=w_sb[:, k, j * P:(j + 1) * P],
                                     rhs=embT[:, k, :],
                                     start=(k == 0 and not has_bd),
                                     stop=(k == D // P - 1))
                nc.vector.tensor_copy(out=vT[:, s0:s0 + ACHUNK, j], in_=acc[:])
            nc.sync.dma_start(vlogT[:, s0:s0 + ACHUNK, :],
                              vT[:, s0:s0 + ACHUNK, :])

        # ---- phase A1: compose-read token chunks ----
        for i in range(A1_CHUNKS):
            a_chunk(i)

        # ---- compose supertiles ----
        idx_off = 0
        for ti, (l, tbase, w) in enumerate(tiles):
            bound = bounds[ti]
            g = sb.tile([P, 4 * w, 2], BF16, tag=f"g{w}", bufs=3)
            nc.gpsimd.ap_gather(
                g[:], vT[:, 0:bound, :],
                rd_sb[:, idx_off:idx_off + 4 * w // 16],
                channels=P, num_elems=bound, d=2, num_idxs=4 * w)
            idx_off += 4 * w // 16
            s01 = sb.tile([P, w, 2], F32, tag=f"s01_{w}")
            nc.vector.tensor_add(out=s01[:], in0=g[:, 0 * w:1 * w, :],
                                 in1=g[:, 1 * w:2 * w, :])
            s23 = sb.tile([P, w, 2], F32, tag=f"s23_{w}")
            nc.vector.tensor_add(out=s23[:], in0=g[:, 2 * w:3 * w, :],
                                 in1=g[:, 3 * w:4 * w, :])
            # final add writes the transposed bf16 mean*4 (scale folded in gelu)
            meanT = sb.tile([P, 2, w], BF16, tag=f"meanT{w}")
            nc.vector.tensor_add(
                out=meanT[:].rearrange("p j w -> p w j"),
                in0=s01[:], in1=s23[:])

            hT = sb.tile([P, HD // P, w], BF16, tag=f"hT{w}", bufs=2)
            for i in range(HD // P):
                phb = ps.tile([P, WTILE], F32, tag="ph", bufs=2)
                ph = phb[:, 0:w]
                if has_b1:
                    nc.tensor.matmul(ph, lhsT=bc1_sb[:, i * P:(i + 1) * P],
                                     rhs=ones1[:, 0:w], start=True, stop=False)
                for k in range(CD // P):
                    nc.tensor.matmul(ph, lhsT=wc1_sb[:, k, i * P:(i + 1) * P],
                                     rhs=meanT[:, k, :],
                                     start=(k == 0 and not has_b1),
                                     stop=(k == CD // P - 1))
                nc.scalar.activation(
                    out=hT[:, i, :], in_=ph,
                    func=mybir.ActivationFunctionType.Gelu_apprx_tanh,
                    scale=float(inv))
            for j in range(CD // P):
                pob = ps.tile([P, WTILE], F32, tag="po", bufs=2)
                po = pob[:, 0:w]
                if has_b2:
                    nc.tensor.matmul(po, lhsT=bc2_sb[:, j * P:(j + 1) * P],
                                     rhs=ones1[:, 0:w], start=True, stop=False)
                for k in range(HD // P):
                    nc.tensor.matmul(po, lhsT=wc2_sb[:, k, j * P:(j + 1) * P],
                                     rhs=hT[:, k, :],
                                     start=(k == 0 and not has_b2),
                                     stop=(k == HD // P - 1))
                nc.vector.tensor_copy(out=vT[:, tbase:tbase + w, j], in_=po)
            nc.sync.dma_start(vlogT[:, tbase:tbase + w, :],
                              vT[:, tbase:tbase + w, :])

        # ---- phase A2: filler token chunks ----
        for i in range(A1_CHUNKS, A_CHUNKS):
            a_chunk(i)

    nc.compile()
    return nc


_CACHE = {}


def _get_bass(key):
    if key not in _CACHE:
        _CACHE[key] = build_bass(*key)
    return _CACHE[key]


def _install_ntff_hook():
    try:
        import antenv.axon_hooks  # noqa: F401
        return
    except ImportError:
        pass
    try:
        import trn_agent_boot.trn_boot as _tb
        hooks = types.ModuleType('antenv.axon_hooks')
        hook = _tb._ntff_profile_via_ctypes('/opt/axon/libaxon_pjrt.so')
        hooks.get_axon_ntff_profile_hook = lambda: hook
        hooks.set_axon_ntff_profile_hook = lambda h: None
        sys.modules['antenv.axon_hooks'] = hooks
    except Exception:
        pass


def run(inputs, trace=False):
    """Returns (full_output, exec_time_ns or None)."""
    inp = {k: (np.asarray(v) if hasattr(v, 'shape') else v)
           for k, v in inputs.items()}
    spans_list = [inp["spans0"], inp["spans1"], inp["spans2"]]
    cores, meta = plan(inp["chunk_input_ids"], spans_list)

    def f32(x):
        return np.ascontiguousarray(x, np.float32)

    b_down = f32(inp["b_down"]).reshape(1, CD)
    bc1 = f32(inp["bc1"]).reshape(1, HD)
    bc2 = f32(inp["bc2"]).reshape(1, CD)
    has_bd = bool(np.any(b_down))
    has_b1 = bool(np.any(bc1))
    has_b2 = bool(np.any(bc2))

    nc = _get_bass((meta["NTOKP"], meta["A1_CHUNKS"], meta["A_CHUNKS"],
                    meta["tiles"], meta["bounds"], meta["nslots"],
                    has_bd, has_b1, has_b2, meta["inv"]))

    emb_bf = np.asarray(inp["emb_table"], np.float32).astype(ml_dtypes.bfloat16)

    def bf16(x):
        return np.ascontiguousarray(
            np.asarray(x, np.float32).astype(ml_dtypes.bfloat16))

    shared = dict(
        w_down=bf16(inp["w_down"]),
        b_down=b_down,
        wc1=bf16(inp["wc1"]),
        bc1=bc1,
        wc2=bf16(inp["wc2"]),
        bc2=bc2,
    )
    NTOKP = meta["NTOKP"]
    in_maps = []
    for c in range(N_CORES):
        core = cores[c]
        m = dict(shared)
        stream = np.zeros((NTOKP, D), ml_dtypes.bfloat16)
        tl = core["tok_ids"]
        if len(tl):
            stream[:len(tl)] = emb_bf[np.asarray(tl, np.int64)]
        m["emb_s"] = stream
        m["rd_idx"] = wrap_idx16(core["rd"])
        in_maps.append(m)

    _install_ntff_hook()
    res = run_bass_kernel_spmd(nc, in_maps, core_ids=list(range(N_CORES)),
                               trace=trace)
    # host assembly: values[s] for slot s on core c = dump[c][:, s, :] -> cd
    vals = np.stack([np.asarray(res.results[c]["vlogT"]).astype(np.float32)
                     for c in range(N_CORES)])          # [8, 128, nslots, 2]
    vals = vals.transpose(0, 2, 3, 1).reshape(N_CORES, meta["nslots"], CD)
    full = vals[meta["pos_core"], meta["pos_slot"]]
    return full.reshape(16, 2048, CD), res.exec_time_ns


def kernel(**inputs):
    out, _ = run(inputs, trace=False)
    return out
